# revision 1
# baseline (speedup 1.0000x reference)
"""DeepseekMoE Trainium2 Bass kernel (8-core expert-parallel).

kernel(**inputs) takes FULL unsharded inputs (as produced by setup_inputs)
and returns the FULL output [1, 2048, 1024] fp32.

Sharding (8 cores):
  - Expert-parallel: 2 of 16 experts per core.
  - Shared expert: F-dim sliced 1408/8=176 per core (partial sums).
  - Router replicated per core.
  - Host: out = x + sum(per-core partials).

Per-core pipeline: PE-transpose x -> xT; router logits (float32r matmuls) +
top-2 (DVE max8) + sigmoid weights; one-hot -> cumsum (DVE scan) -> per-slot
positions; indirect-scatter (token,weight) into a DRAM slot table; shared
expert SwiGLU (F-slice); indirect-gather tokens into per-expert buffers;
expert SwiGLU MLPs; weighted indirect scatter-add combine onto the shared
output base.
"""
import numpy as np

# ---- problem constants (hardcoded; kernel.py must be self-contained) ----
N = 2048          # tokens
D = 1024          # model dim
E = 16            # experts
F = 1408          # expert ffn dim
C = 320           # per-expert capacity = ceil(1.25 * N*K / E)
NCORES = 8
EPC = E // NCORES  # experts per core = 2
FSH = F // NCORES  # shared-expert F slice = 176
P = 128
NT = N // P        # 16 token tiles
DC = D // P        # 8 d-chunks
FC = F // P        # 11 f-chunks
CP = 384           # padded per-expert slot stride (3*128)
NSLOT = EPC * CP   # 768 slots per core
TRASH = NSLOT      # trash row in meta table
ST = NSLOT // P    # 6 slot tiles

_BUILD_CACHE = {}


def _build_nc(debug=False, stage=99):
    import concourse.bacc as bacc
    import concourse.bass as bass
    import concourse.mybir as mybir
    import concourse.tile as tile
    from concourse.masks import make_identity

    f32 = mybir.dt.float32
    f32r = mybir.dt.float32r
    i32 = mybir.dt.int32
    u32 = mybir.dt.uint32
    Alu = mybir.AluOpType
    Act = mybir.ActivationFunctionType

    nc = bacc.Bacc("TRN2", target_bir_lowering=False, debug=False)

    # ---- I/O ----
    x_r = nc.dram_tensor("x_r", [N, D], f32, kind="ExternalInput").ap()
    rw = nc.dram_tensor("rw", [D, E], f32, kind="ExternalInput").ap()
    rb = nc.dram_tensor("rb", [E, 1], f32, kind="ExternalInput").ap()
    wg = nc.dram_tensor("wg", [EPC, D, F], f32r, kind="ExternalInput").ap()
    wu = nc.dram_tensor("wu", [EPC, D, F], f32r, kind="ExternalInput").ap()
    wd = nc.dram_tensor("wd", [EPC, F, D], f32r, kind="ExternalInput").ap()
    swg = nc.dram_tensor("swg", [D, FSH], f32r, kind="ExternalInput").ap()
    swu = nc.dram_tensor("swu", [D, FSH], f32r, kind="ExternalInput").ap()
    swd = nc.dram_tensor("swd", [FSH, D], f32r, kind="ExternalInput").ap()
    coff = nc.dram_tensor("coff", [P, 1], f32, kind="ExternalInput").ap()
    partial = nc.dram_tensor("partial", [N, D], f32, kind="ExternalOutput").ap()
    meta = nc.dram_tensor("meta", [NSLOT + 1, 2], f32, kind="Internal").ap()
    if debug:
        dbg_route = nc.dram_tensor("dbg_route", [N, 8], f32, kind="ExternalOutput").ap()
        dbg_meta = nc.dram_tensor("dbg_meta", [NSLOT + 1, 2], f32, kind="ExternalOutput").ap()

    FSH_CH = [(0, P), (P, FSH - P)]       # shared-expert f chunks: 128 + 48
    FG = [(0, 3), (3, 3), (6, 3), (9, 2)]  # expert f-chunk groups

    with tile.TileContext(nc) as tc:
        with tc.tile_pool(name="persist", bufs=1) as pp, \
             tc.tile_pool(name="stream", bufs=3) as sp:

            # ================= constants =================
            ident = pp.tile([P, P], f32, tag="ident")
            make_identity(nc, ident[:])
            ident_r = pp.tile([P, P], f32r, tag="ident_r")
            nc.vector.tensor_copy(ident_r[:], ident[:])
            iota_e = pp.tile([P, E], i32, tag="iota_e")
            nc.gpsimd.iota(iota_e[:], pattern=[[1, E]], base=0, channel_multiplier=0)
            iota_f = pp.tile([P, E], f32, tag="iota_f")
            nc.vector.tensor_copy(iota_f[:], iota_e[:])
            tokp_i = pp.tile([P, 1], i32, tag="tokp_i")
            nc.gpsimd.iota(tokp_i[:], pattern=[[1, 1]], base=0, channel_multiplier=1)
            tokp_f = pp.tile([P, 1], f32, tag="tokp_f")
            nc.vector.tensor_copy(tokp_f[:], tokp_i[:])
            coff_t = pp.tile([P, 1], f32, tag="coff_t")
            nc.sync.dma_start(coff_t[:], coff)
            zmeta = pp.tile([P, 2], f32, tag="zmeta")
            nc.vector.memset(zmeta[:], 0.0)
            for c in range(ST):
                nc.sync.dma_start(meta[c * P:(c + 1) * P, :], zmeta[:])
            nc.sync.dma_start(meta[NSLOT:NSLOT + 1, :], zmeta[:1, :])

            # routing staging [128, NT] (column = token tile)
            d01s = pp.tile([P, NT], f32, tag="d01s")
            idx0s = pp.tile([P, NT], f32, tag="idx0s")
            idx1s = pp.tile([P, NT], f32, tag="idx1s")
            pos0s = pp.tile([P, NT], f32, tag="pos0s")
            pos1s = pp.tile([P, NT], f32, tag="pos1s")

            with tc.tile_pool(name="pxT", bufs=1) as pxp:
                xT = [pxp.tile([P, N], f32r, tag=f"xT_{d}", name=f"xT_{d}") for d in range(DC)]

                # ======== phase B+C: transpose x (fp32 exact), fp32 router ========
                rwt = []
                for d in range(DC):
                    rwd = pp.tile([P, E], f32, tag=f"rw_{d}")
                    nc.sync.dma_start(rwd[:], rw[d * P:(d + 1) * P, :])
                    rwt.append(rwd)
                rbt = pp.tile([E, 1], f32, tag="rbt")
                nc.sync.dma_start(rbt[:], rb)
                with tc.tile_pool(name="pRt", bufs=1) as prt:
                    lgT = prt.tile([E, N], f32, tag="lgT")
                    eq0s, eq1s = [], []
                    with tc.tile_pool(name="pBx", bufs=6) as pbx, \
                         tc.tile_pool(name="pBp", bufs=1, space="PSUM") as pbp:
                        for tg in range(NT // 4):
                            xtiles = []
                            for ti in range(4):
                                xt = pbx.tile([P, D], f32, tag="xload", bufs=4)
                                nc.sync.dma_start(
                                    xt[:], x_r[(tg * 4 + ti) * P:(tg * 4 + ti + 1) * P, :])
                                xtiles.append(xt)
                            xTf = []
                            for d in range(DC):
                                ps = pbp.tile([P, 4 * P], f32, space="PSUM",
                                              tag="ptx", bufs=3)
                                for ti in range(4):
                                    nc.tensor.transpose(
                                        ps[:, ti * P:(ti + 1) * P],
                                        xtiles[ti][:, d * P:(d + 1) * P],
                                        ident[:])
                                nc.scalar.copy(
                                    xT[d][:, tg * 4 * P:(tg + 1) * 4 * P], ps[:])
                                xf = pbx.tile([P, 4 * P], f32, tag="xTf", bufs=8)
                                nc.vector.tensor_copy(xf[:], ps[:])
                                xTf.append(xf)
                            psl = pbp.tile([E, 512], f32, space="PSUM",
                                           tag="ps_lg", bufs=2)
                            for d in range(DC):
                                nc.tensor.matmul(psl[:], rwt[d][:], xTf[d][:],
                                                 start=(d == 0), stop=(d == DC - 1))
                            nc.scalar.activation(
                                lgT[:, tg * 512:(tg + 1) * 512], psl[:],
                                Act.Identity, bias=rbt[:, 0:1], scale=1.0)

                    # ====== phases D/E: top-2, weights, one-hot ======
                    with tc.tile_pool(name="pRt2", bufs=1) as prt2, \
                         tc.tile_pool(name="pRp", bufs=3, space="PSUM") as prp:
                        ohT = prt2.tile([E, N], f32, tag="ohT")
                        cum = prt2.tile([E, N], f32, tag="cum")
                        for t in range(NT):
                            psl = prp.tile([P, E], f32, space="PSUM", tag="ps_r")
                            nc.tensor.transpose(psl[:], lgT[:, t * P:(t + 1) * P],
                                                ident[:E, :E])
                            lg_t = sp.tile([P, E], f32, tag="lg_t")
                            nc.vector.tensor_copy(lg_t[:], psl[:])
                            mx = sp.tile([P, 8], f32, tag="mx")
                            mi = sp.tile([P, 8], u32, tag="mi")
                            nc.vector.max(mx[:], lg_t[:])
                            nc.vector.max_index(mi[:], mx[:], lg_t[:])
                            nc.vector.tensor_tensor(out=d01s[:, t:t + 1], in0=mx[:, 0:1],
                                                    in1=mx[:, 1:2], op=Alu.subtract)
                            nc.vector.tensor_copy(idx0s[:, t:t + 1], mi[:, 0:1])
                            nc.vector.tensor_copy(idx1s[:, t:t + 1], mi[:, 1:2])
                            mif = sp.tile([P, 2], f32, tag="mif")
                            nc.vector.tensor_copy(mif[:], mi[:, 0:2])
                            eq0 = pp.tile([P, E], f32, tag=f"eq0_{t}")
                            eq1 = pp.tile([P, E], f32, tag=f"eq1_{t}")
                            nc.vector.tensor_tensor(out=eq0[:], in0=iota_f[:],
                                                    in1=mif[:, 0:1].to_broadcast([P, E]),
                                                    op=Alu.is_equal)
                            nc.vector.tensor_tensor(out=eq1[:], in0=iota_f[:],
                                                    in1=mif[:, 1:2].to_broadcast([P, E]),
                                                    op=Alu.is_equal)
                            eq0s.append(eq0)
                            eq1s.append(eq1)
                            oh = sp.tile([P, E], f32, tag="oh")
                            nc.vector.tensor_add(oh[:], eq0[:], eq1[:])
                            pso = prp.tile([E, P], f32, space="PSUM", tag="ps_r")
                            nc.tensor.transpose(pso[:], oh[:], ident[:])
                            nc.scalar.copy(ohT[:, t * P:(t + 1) * P], pso[:])

                        w0s = pp.tile([P, NT], f32, tag="w0s")
                        w1s = pp.tile([P, NT], f32, tag="w1s")
                        nc.scalar.activation(w0s[:], d01s[:], Act.Sigmoid)
                        nc.vector.tensor_scalar(out=w1s[:], in0=w0s[:], scalar1=-1.0,
                                                scalar2=1.0, op0=Alu.mult, op1=Alu.add)

                        # ====== phase F: cumulative counts (inclusive) ======
                        zcol = pp.tile([P, 1], f32, tag="zcol")
                        nc.vector.memset(zcol[:], 0.0)
                        nc.vector.tensor_tensor_scan(
                            cum[:], ohT[:], zcol[:E, 0:1].to_broadcast([E, N]), 0.0,
                            op0=Alu.add, op1=Alu.add)

                        # ====== phase G: per-slot positions ======
                        for t in range(NT):
                            psc = prp.tile([P, E], f32, space="PSUM", tag="ps_r")
                            nc.tensor.transpose(psc[:], cum[:, t * P:(t + 1) * P],
                                                ident[:E, :E])
                            cumP = sp.tile([P, E], f32, tag="cumP")
                            nc.vector.tensor_copy(cumP[:], psc[:])
                            scr = sp.tile([P, E], f32, tag="scr")
                            nc.vector.tensor_mul(scr[:], eq0s[t][:], cumP[:])
                            nc.vector.reduce_sum(pos0s[:, t:t + 1], scr[:],
                                                 axis=mybir.AxisListType.X)
                            scr2 = sp.tile([P, E], f32, tag="scr2")
                            nc.vector.tensor_mul(scr2[:], eq1s[t][:], cumP[:])
                            nc.vector.reduce_sum(pos1s[:, t:t + 1], scr2[:],
                                                 axis=mybir.AxisListType.X)

                # ====== phase H: slot indices ======
                gidx_i = []
                for kk, (idxs, poss) in enumerate(((idx0s, pos0s), (idx1s, pos1s))):
                    loc = pp.tile([P, NT], f32, tag=f"loc{kk}")
                    nc.vector.tensor_scalar(out=loc[:], in0=idxs[:],
                                            scalar1=coff_t[:, 0:1], scalar2=None,
                                            op0=Alu.subtract)
                    pm1 = pp.tile([P, NT], f32, tag=f"pm1{kk}")
                    nc.vector.tensor_scalar_add(pm1[:], poss[:], -1.0)
                    v1 = sp.tile([P, NT], f32, tag="v1")
                    nc.vector.tensor_scalar(out=v1[:], in0=loc[:], scalar1=-0.5,
                                            scalar2=None, op0=Alu.is_gt)
                    v2 = sp.tile([P, NT], f32, tag="v2")
                    nc.vector.tensor_scalar(out=v2[:], in0=loc[:],
                                            scalar1=float(EPC) - 0.5, scalar2=None,
                                            op0=Alu.is_lt)
                    v3 = sp.tile([P, NT], f32, tag="v3")
                    nc.vector.tensor_scalar(out=v3[:], in0=pm1[:],
                                            scalar1=float(C) - 0.5, scalar2=None,
                                            op0=Alu.is_lt)
                    val = pp.tile([P, NT], f32, tag=f"val{kk}")
                    nc.vector.tensor_mul(val[:], v1[:], v2[:])
                    nc.vector.tensor_mul(val[:], val[:], v3[:])
                    gf = pp.tile([P, NT], f32, tag=f"gf{kk}")
                    nc.vector.tensor_scalar(out=gf[:], in0=loc[:], scalar1=float(CP),
                                            scalar2=None, op0=Alu.mult)
                    nc.vector.tensor_add(gf[:], gf[:], pm1[:])
                    nc.vector.tensor_scalar_add(gf[:], gf[:], -float(TRASH))
                    nc.vector.tensor_mul(gf[:], gf[:], val[:])
                    nc.vector.tensor_scalar_add(gf[:], gf[:], float(TRASH))
                    gi = pp.tile([P, NT], i32, tag=f"gi{kk}")
                    nc.vector.tensor_copy(gi[:], gf[:])
                    gidx_i.append(gi)

                if debug:
                    for t in range(NT):
                        dbt = sp.tile([P, 8], f32, tag="dbt")
                        for j, src in enumerate((idx0s, idx1s, pos0s, pos1s, w0s, w1s)):
                            nc.vector.tensor_copy(dbt[:, j:j + 1], src[:, t:t + 1])
                        nc.vector.tensor_copy(dbt[:, 6:7], gidx_i[0][:, t:t + 1])
                        nc.vector.tensor_copy(dbt[:, 7:8], gidx_i[1][:, t:t + 1])
                        nc.sync.dma_start(dbg_route[t * P:(t + 1) * P, :], dbt[:])

                if stage >= 1:
                    # ====== phase I: scatter (token, weight) into meta ======
                    for t in range(NT):
                        for kk, ws in ((0, w0s), (1, w1s)):
                            pay = sp.tile([P, 2], f32, tag="pay")
                            nc.vector.tensor_scalar_add(pay[:, 0:1], tokp_f[:], float(t * P))
                            nc.vector.tensor_copy(pay[:, 1:2], ws[:, t:t + 1])
                            nc.gpsimd.indirect_dma_start(
                                out=meta,
                                out_offset=bass.IndirectOffsetOnAxis(
                                    ap=gidx_i[kk][:, t:t + 1], axis=0),
                                in_=pay[:], in_offset=None,
                                bounds_check=TRASH, oob_is_err=False)

                    # ====== phase J: reload slot tables ======
                    mload = pp.tile([P, ST, 2], f32, tag="mload")
                    nc.sync.dma_start(
                        mload[:], meta[0:NSLOT].rearrange("(c p) w -> p c w", p=P))
                    toki, wcol = [], []
                    for c in range(ST):
                        tci = pp.tile([P, 1], i32, tag=f"toki_{c}")
                        nc.vector.tensor_copy(tci[:], mload[:, c, 0:1])
                        toki.append(tci)
                        wc = pp.tile([P, 1], f32, tag=f"wcol_{c}")
                        nc.vector.tensor_copy(wc[:], mload[:, c, 1:2])
                        wcol.append(wc)
                    if debug:
                        mdump = pp.tile([P, ST, 2], f32, tag="mdump")
                        nc.sync.dma_start(
                            mdump[:], meta[0:NSLOT].rearrange("(c p) w -> p c w", p=P))
                        nc.sync.dma_start(
                            dbg_meta[0:NSLOT].rearrange("(c p) w -> p c w", p=P),
                            mdump[:])

                if stage >= 2:
                        # ====== shared expert: gate/up (uses xT) ======
                    swgt, swut, swdt = [], [], []
                    for d in range(DC):
                        sg_ = pp.tile([P, FSH], f32r, tag=f"swg_{d}")
                        nc.sync.dma_start(sg_[:], swg[d * P:(d + 1) * P, :])
                        swgt.append(sg_)
                        su_ = pp.tile([P, FSH], f32r, tag=f"swu_{d}")
                        nc.sync.dma_start(su_[:], swu[d * P:(d + 1) * P, :])
                        swut.append(su_)
                    for (f0, fl) in FSH_CH:
                        sd_ = pp.tile([fl, D], f32r, tag=f"swd_{f0}")
                        nc.sync.dma_start(sd_[:], swd[f0:f0 + fl, :])
                        swdt.append(sd_)
                    act_sh = [pp.tile([fl, N], f32r, tag=f"actsh_{f0}", name=f"actsh_{f0}")
                              for (f0, fl) in FSH_CH]
                    with tc.tile_pool(name="pSp", bufs=3, space="PSUM") as psp:
                        for fi, (f0, fl) in enumerate(FSH_CH):
                            for n in range(4):
                                psg = psp.tile([P, 512], f32, space="PSUM", tag="ps_sh")
                                psu = psp.tile([P, 512], f32, space="PSUM", tag="ps_sh")
                                for d in range(DC):
                                    nc.tensor.matmul(psg[:fl, :], swgt[d][:, f0:f0 + fl],
                                                     xT[d][:, n * 512:(n + 1) * 512],
                                                     start=(d == 0), stop=(d == DC - 1))
                                for d in range(DC):
                                    nc.tensor.matmul(psu[:fl, :], swut[d][:, f0:f0 + fl],
                                                     xT[d][:, n * 512:(n + 1) * 512],
                                                     start=(d == 0), stop=(d == DC - 1))
                                sgact = sp.tile([P, 512], f32r, tag="sgact", bufs=2)
                                nc.scalar.activation(sgact[:fl, :], psg[:fl, :], Act.Silu)
                                nc.vector.tensor_tensor(
                                    out=act_sh[fi][:, n * 512:(n + 1) * 512],
                                    in0=sgact[:fl, :], in1=psu[:fl, :], op=Alu.mult)
                        # xT no longer needed below this point (pxT closes after)

                        # ====== shared down (token-major) -> base partial ======
                        for t in range(NT):
                            ysh = sp.tile([P, D], f32, tag="big4k", bufs=4, name="ysh")
                            for n in range(2):
                                psd = psp.tile([P, 512], f32, space="PSUM", tag="ps_sh")
                                for fi, (f0, fl) in enumerate(FSH_CH):
                                    nc.tensor.matmul(
                                        psd[:], act_sh[fi][:, t * P:(t + 1) * P],
                                        swdt[fi][:, n * 512:(n + 1) * 512],
                                        start=(fi == 0), stop=(fi == len(FSH_CH) - 1))
                                nc.scalar.copy(ysh[:, n * 512:(n + 1) * 512], psd[:])
                            nc.sync.dma_start(partial[t * P:(t + 1) * P, :], ysh[:])

            if stage >= 3:
                    # ====== dispatch: gather tokens, transpose to hT ======
                hT = [pp.tile([P, NSLOT], f32r, tag=f"hT_{d}", name=f"hT_{d}") for d in range(DC)]
                with tc.tile_pool(name="pthp", bufs=2, space="PSUM") as pth:
                    for c in range(ST):
                        hc = sp.tile([P, D], f32r, tag="big4k", bufs=4, name="hc")
                        nc.gpsimd.indirect_dma_start(
                            out=hc[:], out_offset=None, in_=x_r,
                            in_offset=bass.IndirectOffsetOnAxis(ap=toki[c][:, 0:1], axis=0))
                        for d in range(DC):
                            psh = pth.tile([P, P], f32r, space="PSUM", tag="psh")
                            nc.tensor.transpose(psh[:], hc[:, d * P:(d + 1) * P], ident_r[:])
                            nc.scalar.copy(hT[d][:, c * P:(c + 1) * P], psh[:])

            if stage >= 4:
                    # ====== expert MLPs + combine ======
                with tc.tile_pool(name="wpool", bufs=10) as wp, \
                     tc.tile_pool(name="pact", bufs=11) as pact, \
                     tc.tile_pool(name="pEp", bufs=2, space="PSUM") as pep:
                    for e in range(EPC):
                        acts = []
                        for (fg0, fgn) in FG:
                            wgt, wut = [], []
                            for d in range(DC):
                                wgt_d = wp.tile([P, fgn * P], f32r, tag="wgt", bufs=9)
                                nc.sync.dma_start(
                                    wgt_d[:, :], wg[e, d * P:(d + 1) * P,
                                                    fg0 * P:(fg0 + fgn) * P])
                                wgt.append(wgt_d)
                                wut_d = wp.tile([P, fgn * P], f32r, tag="wut", bufs=9)
                                nc.sync.dma_start(
                                    wut_d[:, :], wu[e, d * P:(d + 1) * P,
                                                    fg0 * P:(fg0 + fgn) * P])
                                wut.append(wut_d)
                            for fi in range(fgn):
                                psg = pep.tile([P, CP], f32, space="PSUM", tag="ps_eg")
                                psu = pep.tile([P, CP], f32, space="PSUM", tag="ps_eu")
                                for d in range(DC):
                                    nc.tensor.matmul(psg[:], wgt[d][:, fi * P:(fi + 1) * P],
                                                     hT[d][:, e * CP:(e + 1) * CP],
                                                     start=(d == 0), stop=(d == DC - 1))
                                for d in range(DC):
                                    nc.tensor.matmul(psu[:], wut[d][:, fi * P:(fi + 1) * P],
                                                     hT[d][:, e * CP:(e + 1) * CP],
                                                     start=(d == 0), stop=(d == DC - 1))
                                sgact = sp.tile([P, CP], f32r, tag="esg", bufs=2)
                                nc.scalar.activation(sgact[:], psg[:], Act.Silu)
                                af = pact.tile([P, CP], f32r, tag="act")
                                nc.vector.tensor_tensor(out=af[:], in0=sgact[:],
                                                        in1=psu[:], op=Alu.mult)
                                acts.append(af)
                        # down-projection (slot-major out) + per-slot weighting
                        ytiles = [sp.tile([P, D], f32, tag="ycomb", bufs=3, name="ycomb")
                                  for _ in range(3)]
                        for n in range(2):
                            wdt = []
                            for f in range(FC):
                                wdt_f = wp.tile([P, 512], f32r, tag="wdt", bufs=11)
                                nc.sync.dma_start(
                                    wdt_f[:], wd[e, f * P:(f + 1) * P,
                                                 n * 512:(n + 1) * 512])
                                wdt.append(wdt_f)
                            for c in range(3):
                                psy = pep.tile([P, 512], f32, space="PSUM", tag="ps_ey")
                                for f in range(FC):
                                    nc.tensor.matmul(psy[:], acts[f][:, c * P:(c + 1) * P],
                                                     wdt[f][:],
                                                     start=(f == 0), stop=(f == FC - 1))
                                nc.vector.tensor_scalar(
                                    out=ytiles[c][:, n * 512:(n + 1) * 512], in0=psy[:],
                                    scalar1=wcol[e * 3 + c][:, 0:1], scalar2=None,
                                    op0=Alu.mult)
                        # combine: scatter-add weighted outputs onto base
                        for c in range(3):
                            nc.gpsimd.indirect_dma_start(
                                out=partial,
                                out_offset=bass.IndirectOffsetOnAxis(
                                    ap=toki[e * 3 + c][:, 0:1], axis=0),
                                in_=ytiles[c][:], in_offset=None,
                                bounds_check=N - 1, oob_is_err=False,
                                compute_op=Alu.add)

    return _finish(nc)


def _finish(nc):
    nc.compile()
    return nc


def _get_nc(debug=False, stage=99):
    key = ("nc", debug, stage)
    if key not in _BUILD_CACHE:
        _BUILD_CACHE[key] = _build_nc(debug, stage)
    return _BUILD_CACHE[key]


def kernel(x, router_w, router_b, w_gate, w_up, w_down, sw_gate, sw_up, sw_down,
           _debug=False, _trace=False):
    from concourse.bass_utils import run_bass_kernel_spmd

    x = np.asarray(x, np.float32)
    x2 = np.ascontiguousarray(x.reshape(N, D))
    in_maps = []
    for m in range(NCORES):
        fs = slice(m * FSH, (m + 1) * FSH)
        in_maps.append({
            "x_r": x2,
            "rw": np.ascontiguousarray(np.asarray(router_w, np.float32)),
            "rb": np.ascontiguousarray(np.asarray(router_b, np.float32).reshape(E, 1)),
            "wg": np.ascontiguousarray(np.asarray(w_gate, np.float32)[m * EPC:(m + 1) * EPC]),
            "wu": np.ascontiguousarray(np.asarray(w_up, np.float32)[m * EPC:(m + 1) * EPC]),
            "wd": np.ascontiguousarray(np.asarray(w_down, np.float32)[m * EPC:(m + 1) * EPC]),
            "swg": np.ascontiguousarray(np.asarray(sw_gate, np.float32)[:, fs]),
            "swu": np.ascontiguousarray(np.asarray(sw_up, np.float32)[:, fs]),
            "swd": np.ascontiguousarray(np.asarray(sw_down, np.float32)[fs, :]),
            "coff": np.full((P, 1), float(m * EPC), np.float32),
        })

    nc = _get_nc(_debug)
    res = run_bass_kernel_spmd(nc, in_maps, core_ids=list(range(NCORES)),
                               trace=_trace)
    out = x2.copy()
    for r in res.results:
        out += r["partial"]
    if _debug or _trace:
        kernel._last_results = res
    return out.reshape(x.shape)



# revision 10
# speedup vs baseline: 2.4123x; 2.4123x over previous
"""DeepseekMoE Trainium2 Bass kernel (8-core expert-parallel).

kernel(**inputs) takes FULL unsharded inputs (as produced by setup_inputs)
and returns the FULL output [1, 2048, 1024] fp32.

Sharding (8 cores):
  - Expert-parallel: 2 of 16 experts per core.
  - Shared expert: F-dim sliced 1408/8=176 per core (partial sums).
  - Router replicated per core.
  - Host: out = x + sum(per-core partials).

Per-core pipeline: PE-transpose x -> xT; router logits (float32r matmuls) +
top-2 (DVE max8) + sigmoid weights; one-hot -> cumsum (DVE scan) -> per-slot
positions; indirect-scatter (token,weight) into a DRAM slot table; shared
expert SwiGLU (F-slice); indirect-gather tokens into per-expert buffers;
expert SwiGLU MLPs; weighted indirect scatter-add combine onto the shared
output base.
"""
import numpy as np

# ---- problem constants (hardcoded; kernel.py must be self-contained) ----
N = 2048          # tokens
D = 1024          # model dim
E = 16            # experts
F = 1408          # expert ffn dim
C = 320           # per-expert capacity = ceil(1.25 * N*K / E)
NCORES = 8
EPC = E // NCORES  # experts per core = 2
FSH = F // NCORES  # shared-expert F slice = 176
P = 128
NT = N // P        # 16 token tiles
DC = D // P        # 8 d-chunks
FC = F // P        # 11 f-chunks
CP = 384           # padded per-expert slot stride (3*128)
NSLOT = EPC * CP   # 768 slots per core
TRASH = NSLOT      # trash row in meta table
ST = NSLOT // P    # 6 slot tiles

_BUILD_CACHE = {}


def _build_nc(debug=False, stage=99):
    import concourse.bacc as bacc
    import concourse.bass as bass
    import concourse.mybir as mybir
    import concourse.tile as tile
    from concourse.masks import make_identity

    f32 = mybir.dt.float32
    f32r = mybir.dt.float32r
    i32 = mybir.dt.int32
    u32 = mybir.dt.uint32
    Alu = mybir.AluOpType
    Act = mybir.ActivationFunctionType

    nc = bacc.Bacc("TRN2", target_bir_lowering=False, debug=False)

    # ---- I/O ----
    x_r = nc.dram_tensor("x_r", [N, D], f32, kind="ExternalInput").ap()
    rw = nc.dram_tensor("rw", [D, E], f32, kind="ExternalInput").ap()
    rb = nc.dram_tensor("rb", [E, 1], f32, kind="ExternalInput").ap()
    wg = nc.dram_tensor("wg", [EPC, D, F], f32r, kind="ExternalInput").ap()
    wu = nc.dram_tensor("wu", [EPC, D, F], f32r, kind="ExternalInput").ap()
    wd = nc.dram_tensor("wd", [EPC, F, D], f32r, kind="ExternalInput").ap()
    swg = nc.dram_tensor("swg", [D, FSH], f32r, kind="ExternalInput").ap()
    swu = nc.dram_tensor("swu", [D, FSH], f32r, kind="ExternalInput").ap()
    swd = nc.dram_tensor("swd", [FSH, D], f32r, kind="ExternalInput").ap()
    coff = nc.dram_tensor("coff", [P, 1], f32, kind="ExternalInput").ap()
    partial = nc.dram_tensor("partial", [N, D], f32, kind="ExternalOutput").ap()
    if debug:
        dbg_route = nc.dram_tensor("dbg_route", [N, 8], f32, kind="ExternalOutput").ap()
        dbg_tbl = nc.dram_tensor("dbg_tbl", [NSLOT, 2], f32, kind="ExternalOutput").ap()

    FSH_CH = [(0, P), (P, FSH - P)]       # shared-expert f chunks: 128 + 48
    FG = [(0, 3), (3, 3), (6, 3), (9, 2)]  # expert f-chunk groups

    with tile.TileContext(nc) as tc:
        with tc.tile_pool(name="persist", bufs=1) as pp, \
             tc.tile_pool(name="stream", bufs=3) as sp:

            # ================= constants =================
            ident = pp.tile([P, P], f32, tag="ident")
            make_identity(nc, ident[:])
            ident_r = pp.tile([P, P], f32r, tag="ident_r")
            nc.vector.tensor_copy(ident_r[:], ident[:])
            iota_e = pp.tile([P, E], i32, tag="iota_e")
            nc.gpsimd.iota(iota_e[:], pattern=[[1, E]], base=0, channel_multiplier=0)
            iota_f = pp.tile([P, E], f32, tag="iota_f")
            nc.vector.tensor_copy(iota_f[:], iota_e[:])
            tokp_i = pp.tile([P, 1], i32, tag="tokp_i")
            nc.gpsimd.iota(tokp_i[:], pattern=[[1, 1]], base=0, channel_multiplier=1)
            tokp_f = pp.tile([P, 1], f32, tag="tokp_f")
            nc.vector.tensor_copy(tokp_f[:], tokp_i[:])
            coff_t = pp.tile([P, 1], f32, tag="coff_t")
            nc.sync.dma_start(coff_t[:], coff)

            # routing staging [128, NT] (column = token tile)
            d01s = pp.tile([P, NT], f32, tag="d01s")
            idx0s = pp.tile([P, NT], f32, tag="idx0s")
            idx1s = pp.tile([P, NT], f32, tag="idx1s")
            pos0s = pp.tile([P, NT], f32, tag="pos0s")
            pos1s = pp.tile([P, NT], f32, tag="pos1s")

            with tc.tile_pool(name="pxT", bufs=1) as pxp:
                xT = [pxp.tile([P, N], f32r, tag=f"xT_{d}", name=f"xT_{d}") for d in range(DC)]

                # ======== phase B+C: transpose x (fp32 exact), fp32 router ========
                rwt = []
                for d in range(DC):
                    rwd = pp.tile([P, E], f32, tag=f"rw_{d}")
                    nc.sync.dma_start(rwd[:], rw[d * P:(d + 1) * P, :])
                    rwt.append(rwd)
                rbt = pp.tile([E, 1], f32, tag="rbt")
                nc.sync.dma_start(rbt[:], rb)
                with tc.tile_pool(name="pRt", bufs=1) as prt:
                    lgT = prt.tile([E, N], f32, tag="lgT")
                    eq0s, eq1s = [], []
                    with tc.tile_pool(name="pBx", bufs=6) as pbx, \
                         tc.tile_pool(name="pBp", bufs=1, space="PSUM") as pbp:
                        for tg in range(NT // 4):
                            xtiles = []
                            for ti in range(4):
                                xt = pbx.tile([P, D], f32, tag="xload", bufs=4)
                                nc.sync.dma_start(
                                    xt[:], x_r[(tg * 4 + ti) * P:(tg * 4 + ti + 1) * P, :])
                                xtiles.append(xt)
                            xTf = []
                            for d in range(DC):
                                ps = pbp.tile([P, 4 * P], f32, space="PSUM",
                                              tag="ptx", bufs=3)
                                for ti in range(4):
                                    nc.tensor.transpose(
                                        ps[:, ti * P:(ti + 1) * P],
                                        xtiles[ti][:, d * P:(d + 1) * P],
                                        ident[:])
                                nc.scalar.copy(
                                    xT[d][:, tg * 4 * P:(tg + 1) * 4 * P], ps[:])
                                xf = pbx.tile([P, 4 * P], f32, tag="xTf", bufs=8)
                                nc.vector.tensor_copy(xf[:], ps[:])
                                xTf.append(xf)
                            psl = pbp.tile([E, 512], f32, space="PSUM",
                                           tag="ps_lg", bufs=2)
                            for d in range(DC):
                                nc.tensor.matmul(psl[:], rwt[d][:], xTf[d][:],
                                                 start=(d == 0), stop=(d == DC - 1))
                            nc.scalar.activation(
                                lgT[:, tg * 512:(tg + 1) * 512], psl[:],
                                Act.Identity, bias=rbt[:, 0:1], scale=1.0)

                    # ====== phases D/E: top-2, weights, one-hot ======
                    with tc.tile_pool(name="pRt2", bufs=1) as prt2, \
                         tc.tile_pool(name="pRp", bufs=3, space="PSUM") as prp:
                        ohT = prt2.tile([E, N], f32, tag="ohT")
                        cum = prt2.tile([E, N], f32, tag="cum")
                        for t in range(NT):
                            psl = prp.tile([P, E], f32, space="PSUM", tag="ps_r")
                            nc.tensor.transpose(psl[:], lgT[:, t * P:(t + 1) * P],
                                                ident[:E, :E])
                            lg_t = sp.tile([P, E], f32, tag="lg_t")
                            nc.vector.tensor_copy(lg_t[:], psl[:])
                            mx = sp.tile([P, 8], f32, tag="mx")
                            mi = sp.tile([P, 8], u32, tag="mi")
                            nc.vector.max(mx[:], lg_t[:])
                            nc.vector.max_index(mi[:], mx[:], lg_t[:])
                            nc.vector.tensor_tensor(out=d01s[:, t:t + 1], in0=mx[:, 0:1],
                                                    in1=mx[:, 1:2], op=Alu.subtract)
                            nc.vector.tensor_copy(idx0s[:, t:t + 1], mi[:, 0:1])
                            nc.vector.tensor_copy(idx1s[:, t:t + 1], mi[:, 1:2])
                            mif = sp.tile([P, 2], f32, tag="mif")
                            nc.vector.tensor_copy(mif[:], mi[:, 0:2])
                            eq0 = pp.tile([P, E], f32, tag=f"eq0_{t}")
                            eq1 = pp.tile([P, E], f32, tag=f"eq1_{t}")
                            nc.vector.tensor_tensor(out=eq0[:], in0=iota_f[:],
                                                    in1=mif[:, 0:1].to_broadcast([P, E]),
                                                    op=Alu.is_equal)
                            nc.vector.tensor_tensor(out=eq1[:], in0=iota_f[:],
                                                    in1=mif[:, 1:2].to_broadcast([P, E]),
                                                    op=Alu.is_equal)
                            eq0s.append(eq0)
                            eq1s.append(eq1)
                            oh = sp.tile([P, E], f32, tag="oh")
                            nc.vector.tensor_add(oh[:], eq0[:], eq1[:])
                            pso = prp.tile([E, P], f32, space="PSUM", tag="ps_r")
                            nc.tensor.transpose(pso[:], oh[:], ident[:])
                            nc.scalar.copy(ohT[:, t * P:(t + 1) * P], pso[:])

                        w0s = pp.tile([P, NT], f32, tag="w0s")
                        w1s = pp.tile([P, NT], f32, tag="w1s")
                        nc.scalar.activation(w0s[:], d01s[:], Act.Sigmoid)
                        nc.vector.tensor_scalar(out=w1s[:], in0=w0s[:], scalar1=-1.0,
                                                scalar2=1.0, op0=Alu.mult, op1=Alu.add)

                        # ====== phase F: cumulative counts (inclusive) ======
                        zcol = pp.tile([P, 1], f32, tag="zcol")
                        nc.vector.memset(zcol[:], 0.0)
                        nc.vector.tensor_tensor_scan(
                            cum[:], ohT[:], zcol[:E, 0:1].to_broadcast([E, N]), 0.0,
                            op0=Alu.add, op1=Alu.add)

                        # ====== phase G: per-slot positions ======
                        for t in range(NT):
                            psc = prp.tile([P, E], f32, space="PSUM", tag="ps_r")
                            nc.tensor.transpose(psc[:], cum[:, t * P:(t + 1) * P],
                                                ident[:E, :E])
                            cumP = sp.tile([P, E], f32, tag="cumP")
                            nc.vector.tensor_copy(cumP[:], psc[:])
                            scr = sp.tile([P, E], f32, tag="scr")
                            nc.vector.tensor_mul(scr[:], eq0s[t][:], cumP[:])
                            nc.vector.reduce_sum(pos0s[:, t:t + 1], scr[:],
                                                 axis=mybir.AxisListType.X)
                            scr2 = sp.tile([P, E], f32, tag="scr2")
                            nc.vector.tensor_mul(scr2[:], eq1s[t][:], cumP[:])
                            nc.vector.reduce_sum(pos1s[:, t:t + 1], scr2[:],
                                                 axis=mybir.AxisListType.X)

                # ====== phase H: slot indices (f32; TRASH=NSLOT matches no slot) ======
                slotf = []
                for kk, (idxs, poss) in enumerate(((idx0s, pos0s), (idx1s, pos1s))):
                    loc = pp.tile([P, NT], f32, tag=f"loc{kk}")
                    nc.vector.tensor_scalar(out=loc[:], in0=idxs[:],
                                            scalar1=coff_t[:, 0:1], scalar2=None,
                                            op0=Alu.subtract)
                    pm1 = pp.tile([P, NT], f32, tag=f"pm1{kk}")
                    nc.vector.tensor_scalar_add(pm1[:], poss[:], -1.0)
                    v1 = sp.tile([P, NT], f32, tag="v1")
                    nc.vector.tensor_scalar(out=v1[:], in0=loc[:], scalar1=-0.5,
                                            scalar2=None, op0=Alu.is_gt)
                    v2 = sp.tile([P, NT], f32, tag="v2")
                    nc.vector.tensor_scalar(out=v2[:], in0=loc[:],
                                            scalar1=float(EPC) - 0.5, scalar2=None,
                                            op0=Alu.is_lt)
                    v3 = sp.tile([P, NT], f32, tag="v3")
                    nc.vector.tensor_scalar(out=v3[:], in0=pm1[:],
                                            scalar1=float(C) - 0.5, scalar2=None,
                                            op0=Alu.is_lt)
                    val = pp.tile([P, NT], f32, tag=f"val{kk}")
                    nc.vector.tensor_mul(val[:], v1[:], v2[:])
                    nc.vector.tensor_mul(val[:], val[:], v3[:])
                    gf = pp.tile([P, NT], f32, tag=f"gf{kk}")
                    nc.vector.tensor_scalar(out=gf[:], in0=loc[:], scalar1=float(CP),
                                            scalar2=None, op0=Alu.mult)
                    nc.vector.tensor_add(gf[:], gf[:], pm1[:])
                    nc.vector.tensor_scalar_add(gf[:], gf[:], -float(TRASH))
                    nc.vector.tensor_mul(gf[:], gf[:], val[:])
                    nc.vector.tensor_scalar_add(gf[:], gf[:], float(TRASH))
                    slotf.append(gf)

                if debug:
                    for t in range(NT):
                        dbt = sp.tile([P, 8], f32, tag="dbt")
                        for j, src in enumerate((idx0s, idx1s, pos0s, pos1s, w0s, w1s)):
                            nc.vector.tensor_copy(dbt[:, j:j + 1], src[:, t:t + 1])
                        nc.vector.tensor_copy(dbt[:, 6:7], slotf[0][:, t:t + 1])
                        nc.vector.tensor_copy(dbt[:, 7:8], slotf[1][:, t:t + 1])
                        nc.sync.dma_start(dbg_route[t * P:(t + 1) * P, :], dbt[:])

                if stage >= 1:
                    # ====== phase I: slot tables via one-hot matmuls ======
                    # mt[p, s] = (slot_k(token) == s); psum rows accumulate
                    # (tile_base, lane, weight) per slot. Token id split keeps
                    # every operand exactly representable at low precision.
                    toki, wcol = [], []
                    with tc.tile_pool(name="pTbl", bufs=1) as ptl, \
                         tc.tile_pool(name="ptb", bufs=1, space="PSUM") as ptb:
                        iota_s_i = ptl.tile([P, NSLOT], i32, tag="iota_s_i")
                        nc.gpsimd.iota(iota_s_i[:], pattern=[[1, NSLOT]], base=0,
                                       channel_multiplier=0)
                        iota_s = ptl.tile([P, NSLOT], f32r, tag="iota_s")
                        nc.vector.tensor_copy(iota_s[:], iota_s_i[:])
                        tbl_ps = [ptb.tile([3, CP], f32, space="PSUM",
                                           tag=f"tbl_ps{h}", name=f"tbl_ps{h}")
                                  for h in range(2)]
                        for t in range(NT):
                            for kk, ws in ((0, w0s), (1, w1s)):
                                pay = ptl.tile([P, 3], f32r, tag="pay", bufs=3)
                                nc.vector.tensor_scalar(
                                    out=pay[:, 0:1], in0=tokp_f[:], scalar1=0.0,
                                    scalar2=float(t * P), op0=Alu.mult,
                                    op1=Alu.add)
                                nc.vector.tensor_copy(pay[:, 1:2], tokp_f[:])
                                nc.vector.tensor_copy(pay[:, 2:3], ws[:, t:t + 1])
                                mt = ptl.tile([P, NSLOT], f32r, tag="mt", bufs=3)
                                nc.vector.tensor_scalar(
                                    out=mt[:], in0=iota_s[:],
                                    scalar1=slotf[kk][:, t:t + 1], scalar2=None,
                                    op0=Alu.is_equal)
                                first = (t == 0 and kk == 0)
                                last = (t == NT - 1 and kk == 1)
                                for h in range(2):
                                    nc.tensor.matmul(
                                        tbl_ps[h][:], pay[:],
                                        mt[:, h * CP:(h + 1) * CP],
                                        start=first, stop=last)
                        tbl_sb = ptl.tile([3, NSLOT], f32, tag="tbl_sb")
                        for h in range(2):
                            nc.scalar.copy(tbl_sb[:, h * CP:(h + 1) * CP],
                                           tbl_ps[h][:])
                        for c in range(ST):
                            pst = ptb.tile([P, 3], f32, space="PSUM", tag="pst",
                                           bufs=2)
                            nc.tensor.transpose(
                                pst[:], tbl_sb[:, c * P:(c + 1) * P],
                                ident[:3, :3])
                            pstv = ptl.tile([P, 3], f32, tag="pstv", bufs=2)
                            nc.vector.tensor_copy(pstv[:], pst[:])
                            tcf = ptl.tile([P, 1], f32, tag="tcf", bufs=2)
                            nc.vector.tensor_tensor(out=tcf[:], in0=pstv[:, 0:1],
                                                    in1=pstv[:, 1:2], op=Alu.add)
                            tci = pp.tile([P, 1], i32, tag=f"toki_{c}")
                            nc.vector.tensor_copy(tci[:], tcf[:])
                            toki.append(tci)
                            wc = pp.tile([P, 1], f32, tag=f"wcol_{c}")
                            nc.vector.tensor_copy(wc[:], pstv[:, 2:3])
                            wcol.append(wc)
                            if debug:
                                dtb = ptl.tile([P, 2], f32, tag="dtb", bufs=2)
                                nc.vector.tensor_copy(dtb[:, 0:1], tcf[:])
                                nc.vector.tensor_copy(dtb[:, 1:2], pstv[:, 2:3])
                                nc.sync.dma_start(
                                    dbg_tbl[c * P:(c + 1) * P, :], dtb[:])

                if stage >= 2:
                        # ====== shared expert: gate/up (uses xT) ======
                    swgt, swut, swdt = [], [], []
                    for d in range(DC):
                        sg_ = pp.tile([P, FSH], f32r, tag=f"swg_{d}")
                        nc.sync.dma_start(sg_[:], swg[d * P:(d + 1) * P, :])
                        swgt.append(sg_)
                        su_ = pp.tile([P, FSH], f32r, tag=f"swu_{d}")
                        nc.sync.dma_start(su_[:], swu[d * P:(d + 1) * P, :])
                        swut.append(su_)
                    for (f0, fl) in FSH_CH:
                        sd_ = pp.tile([fl, D], f32r, tag=f"swd_{f0}")
                        nc.sync.dma_start(sd_[:], swd[f0:f0 + fl, :])
                        swdt.append(sd_)
                    act_sh = [pp.tile([fl, N], f32r, tag=f"actsh_{f0}", name=f"actsh_{f0}")
                              for (f0, fl) in FSH_CH]
                    with tc.tile_pool(name="pSp", bufs=3, space="PSUM") as psp:
                        for fi, (f0, fl) in enumerate(FSH_CH):
                            for n in range(4):
                                psg = psp.tile([P, 512], f32, space="PSUM", tag="ps_sh")
                                psu = psp.tile([P, 512], f32, space="PSUM", tag="ps_sh")
                                for d in range(DC):
                                    nc.tensor.matmul(psg[:fl, :], swgt[d][:, f0:f0 + fl],
                                                     xT[d][:, n * 512:(n + 1) * 512],
                                                     start=(d == 0), stop=(d == DC - 1))
                                for d in range(DC):
                                    nc.tensor.matmul(psu[:fl, :], swut[d][:, f0:f0 + fl],
                                                     xT[d][:, n * 512:(n + 1) * 512],
                                                     start=(d == 0), stop=(d == DC - 1))
                                sgact = sp.tile([P, 512], f32r, tag="sgact", bufs=2)
                                nc.scalar.activation(sgact[:fl, :], psg[:fl, :], Act.Silu)
                                nc.vector.tensor_tensor(
                                    out=act_sh[fi][:, n * 512:(n + 1) * 512],
                                    in0=sgact[:fl, :], in1=psu[:fl, :], op=Alu.mult)
                        # xT no longer needed below this point (pxT closes after)

                        # ====== shared down (token-major) -> base partial ======
                        for t in range(NT):
                            ysh = sp.tile([P, D], f32, tag="big4k", bufs=4, name="ysh")
                            for n in range(2):
                                psd = psp.tile([P, 512], f32, space="PSUM", tag="ps_sh")
                                for fi, (f0, fl) in enumerate(FSH_CH):
                                    nc.tensor.matmul(
                                        psd[:], act_sh[fi][:, t * P:(t + 1) * P],
                                        swdt[fi][:, n * 512:(n + 1) * 512],
                                        start=(fi == 0), stop=(fi == len(FSH_CH) - 1))
                                nc.scalar.copy(ysh[:, n * 512:(n + 1) * 512], psd[:])
                            nc.sync.dma_start(partial[t * P:(t + 1) * P, :], ysh[:])

            if stage >= 3:
                    # ====== dispatch: gather tokens, transpose to hT ======
                hT = [pp.tile([P, NSLOT], f32r, tag=f"hT_{d}", name=f"hT_{d}") for d in range(DC)]
                with tc.tile_pool(name="pthp", bufs=2, space="PSUM") as pth:
                    for c in range(ST):
                        hc = sp.tile([P, D], f32r, tag="big4k", bufs=4, name="hc")
                        nc.gpsimd.indirect_dma_start(
                            out=hc[:], out_offset=None, in_=x_r,
                            in_offset=bass.IndirectOffsetOnAxis(ap=toki[c][:, 0:1], axis=0))
                        for d in range(DC):
                            psh = pth.tile([P, P], f32r, space="PSUM", tag="psh")
                            nc.tensor.transpose(psh[:], hc[:, d * P:(d + 1) * P], ident_r[:])
                            nc.scalar.copy(hT[d][:, c * P:(c + 1) * P], psh[:])

            if stage >= 4:
                    # ====== expert MLPs + combine ======
                with tc.tile_pool(name="wpool", bufs=10) as wp, \
                     tc.tile_pool(name="pact", bufs=11) as pact, \
                     tc.tile_pool(name="pEp", bufs=2, space="PSUM") as pep:
                    for e in range(EPC):
                        acts = []
                        for (fg0, fgn) in FG:
                            wgt, wut = [], []
                            for d in range(DC):
                                wgt_d = wp.tile([P, fgn * P], f32r, tag="wgt", bufs=9)
                                nc.sync.dma_start(
                                    wgt_d[:, :], wg[e, d * P:(d + 1) * P,
                                                    fg0 * P:(fg0 + fgn) * P])
                                wgt.append(wgt_d)
                                wut_d = wp.tile([P, fgn * P], f32r, tag="wut", bufs=9)
                                nc.sync.dma_start(
                                    wut_d[:, :], wu[e, d * P:(d + 1) * P,
                                                    fg0 * P:(fg0 + fgn) * P])
                                wut.append(wut_d)
                            for fi in range(fgn):
                                psg = pep.tile([P, CP], f32, space="PSUM", tag="ps_eg")
                                psu = pep.tile([P, CP], f32, space="PSUM", tag="ps_eu")
                                for d in range(DC):
                                    nc.tensor.matmul(psg[:], wgt[d][:, fi * P:(fi + 1) * P],
                                                     hT[d][:, e * CP:(e + 1) * CP],
                                                     start=(d == 0), stop=(d == DC - 1))
                                for d in range(DC):
                                    nc.tensor.matmul(psu[:], wut[d][:, fi * P:(fi + 1) * P],
                                                     hT[d][:, e * CP:(e + 1) * CP],
                                                     start=(d == 0), stop=(d == DC - 1))
                                sgact = sp.tile([P, CP], f32r, tag="esg", bufs=2)
                                nc.scalar.activation(sgact[:], psg[:], Act.Silu)
                                af = pact.tile([P, CP], f32r, tag="act")
                                nc.vector.tensor_tensor(out=af[:], in0=sgact[:],
                                                        in1=psu[:], op=Alu.mult)
                                acts.append(af)
                        # down-projection (slot-major out) + per-slot weighting
                        ytiles = [sp.tile([P, D], f32, tag="ycomb", bufs=3, name="ycomb")
                                  for _ in range(3)]
                        for n in range(2):
                            wdt = []
                            for f in range(FC):
                                wdt_f = wp.tile([P, 512], f32r, tag="wdt", bufs=11)
                                nc.sync.dma_start(
                                    wdt_f[:], wd[e, f * P:(f + 1) * P,
                                                 n * 512:(n + 1) * 512])
                                wdt.append(wdt_f)
                            for c in range(3):
                                psy = pep.tile([P, 512], f32, space="PSUM", tag="ps_ey")
                                for f in range(FC):
                                    nc.tensor.matmul(psy[:], acts[f][:, c * P:(c + 1) * P],
                                                     wdt[f][:],
                                                     start=(f == 0), stop=(f == FC - 1))
                                nc.vector.tensor_scalar(
                                    out=ytiles[c][:, n * 512:(n + 1) * 512], in0=psy[:],
                                    scalar1=wcol[e * 3 + c][:, 0:1], scalar2=None,
                                    op0=Alu.mult)
                        # combine: scatter-add weighted outputs onto base
                        for c in range(3):
                            nc.gpsimd.indirect_dma_start(
                                out=partial,
                                out_offset=bass.IndirectOffsetOnAxis(
                                    ap=toki[e * 3 + c][:, 0:1], axis=0),
                                in_=ytiles[c][:], in_offset=None,
                                bounds_check=N - 1, oob_is_err=False,
                                compute_op=Alu.add)

    return _finish(nc)


def _finish(nc):
    nc.compile()
    return nc


def _get_nc(debug=False, stage=99):
    key = ("nc", debug, stage)
    if key not in _BUILD_CACHE:
        _BUILD_CACHE[key] = _build_nc(debug, stage)
    return _BUILD_CACHE[key]


def kernel(x, router_w, router_b, w_gate, w_up, w_down, sw_gate, sw_up, sw_down,
           _debug=False, _trace=False):
    from concourse.bass_utils import run_bass_kernel_spmd

    x = np.asarray(x, np.float32)
    x2 = np.ascontiguousarray(x.reshape(N, D))
    in_maps = []
    for m in range(NCORES):
        fs = slice(m * FSH, (m + 1) * FSH)
        in_maps.append({
            "x_r": x2,
            "rw": np.ascontiguousarray(np.asarray(router_w, np.float32)),
            "rb": np.ascontiguousarray(np.asarray(router_b, np.float32).reshape(E, 1)),
            "wg": np.ascontiguousarray(np.asarray(w_gate, np.float32)[m * EPC:(m + 1) * EPC]),
            "wu": np.ascontiguousarray(np.asarray(w_up, np.float32)[m * EPC:(m + 1) * EPC]),
            "wd": np.ascontiguousarray(np.asarray(w_down, np.float32)[m * EPC:(m + 1) * EPC]),
            "swg": np.ascontiguousarray(np.asarray(sw_gate, np.float32)[:, fs]),
            "swu": np.ascontiguousarray(np.asarray(sw_up, np.float32)[:, fs]),
            "swd": np.ascontiguousarray(np.asarray(sw_down, np.float32)[fs, :]),
            "coff": np.full((P, 1), float(m * EPC), np.float32),
        })

    nc = _get_nc(_debug)
    res = run_bass_kernel_spmd(nc, in_maps, core_ids=list(range(NCORES)),
                               trace=_trace)
    out = x2.copy()
    for r in res.results:
        out += r["partial"]
    if _debug or _trace:
        kernel._last_results = res
    return out.reshape(x.shape)



# revision 32
# speedup vs baseline: 3.3230x; 1.3776x over previous
"""DeepseekMoE Trainium2 Bass kernel (8-core expert-parallel).

kernel(**inputs) takes FULL unsharded inputs (as produced by setup_inputs)
and returns the FULL output [1, 2048, 1024] fp32.

Sharding (8 cores):
  - Expert-parallel: 2 of 16 experts per core.
  - Shared expert: F-dim sliced 1408/8=176 per core (partial sums).
  - Router replicated per core.
  - Host: out = x + sum(per-core partials).

Per-core pipeline: PE-transpose x -> xT; router logits (float32r matmuls) +
top-2 (DVE max8) + sigmoid weights; one-hot -> cumsum (DVE scan) -> per-slot
positions; indirect-scatter (token,weight) into a DRAM slot table; shared
expert SwiGLU (F-slice); indirect-gather tokens into per-expert buffers;
expert SwiGLU MLPs; weighted indirect scatter-add combine onto the shared
output base.
"""
import numpy as np

# ---- problem constants (hardcoded; kernel.py must be self-contained) ----
N = 2048          # tokens
D = 1024          # model dim
E = 16            # experts
F = 1408          # expert ffn dim
C = 320           # per-expert capacity = ceil(1.25 * N*K / E)
NCORES = 8
EPC = E // NCORES  # experts per core = 2
FSH = F // NCORES  # shared-expert F slice = 176
P = 128
NT = N // P        # 16 token tiles
DC = D // P        # 8 d-chunks
FC = F // P        # 11 f-chunks
CP = 384           # padded per-expert slot stride (3*128)
NSLOT = EPC * CP   # 768 slots per core
TRASH = NSLOT      # trash row in meta table
ST = NSLOT // P    # 6 slot tiles

_BUILD_CACHE = {}


def _build_nc(debug=False, stage=99):
    import concourse.bacc as bacc
    import concourse.bass as bass
    import concourse.mybir as mybir
    import concourse.tile as tile
    from concourse.masks import make_identity

    f32 = mybir.dt.float32
    f32r = mybir.dt.float32r
    f16 = mybir.dt.float16
    bf16 = mybir.dt.bfloat16
    i32 = mybir.dt.int32
    u32 = mybir.dt.uint32
    Alu = mybir.AluOpType
    Act = mybir.ActivationFunctionType

    nc = bacc.Bacc("TRN2", target_bir_lowering=False, debug=False)

    # ---- I/O ----
    x_r = nc.dram_tensor("x_r", [N, D], f32, kind="ExternalInput").ap()
    rw = nc.dram_tensor("rw", [D, E], f32, kind="ExternalInput").ap()
    rb = nc.dram_tensor("rb", [E, 1], f32, kind="ExternalInput").ap()
    wg = nc.dram_tensor("wg", [EPC, D, F], bf16, kind="ExternalInput").ap()
    wu = nc.dram_tensor("wu", [EPC, D, F], bf16, kind="ExternalInput").ap()
    wd = nc.dram_tensor("wd", [EPC, F, D], bf16, kind="ExternalInput").ap()
    swg = nc.dram_tensor("swg", [D, FSH], f32r, kind="ExternalInput").ap()
    swu = nc.dram_tensor("swu", [D, FSH], f32r, kind="ExternalInput").ap()
    swd = nc.dram_tensor("swd", [FSH, D], f32r, kind="ExternalInput").ap()
    coff = nc.dram_tensor("coff", [P, 1], f32, kind="ExternalInput").ap()
    partial = nc.dram_tensor("partial", [N, D], f32, kind="ExternalOutput").ap()
    if debug:
        dbg_route = nc.dram_tensor("dbg_route", [N, 8], f32, kind="ExternalOutput").ap()
        dbg_tbl = nc.dram_tensor("dbg_tbl", [NSLOT, 2], f32, kind="ExternalOutput").ap()

    FSH_CH = [(0, P), (P, FSH - P)]       # shared-expert f chunks: 128 + 48
    FG = [(0, 3), (3, 3), (6, 3), (9, 2)]  # expert f-chunk groups

    with tile.TileContext(nc) as tc:
        with tc.tile_pool(name="persist", bufs=1) as pp, \
             tc.tile_pool(name="stream", bufs=3) as sp:

            # ================= constants =================
            ident = pp.tile([P, P], f32, tag="ident")
            make_identity(nc, ident[:])
            ident_r = pp.tile([P, P], f32r, tag="ident_r")
            nc.vector.tensor_copy(ident_r[:], ident[:])
            iota_e = pp.tile([P, E], i32, tag="iota_e")
            nc.gpsimd.iota(iota_e[:], pattern=[[1, E]], base=0, channel_multiplier=0)
            iota_f = pp.tile([P, E], f32, tag="iota_f")
            nc.vector.tensor_copy(iota_f[:], iota_e[:])
            tokp_i = pp.tile([P, 1], i32, tag="tokp_i")
            nc.gpsimd.iota(tokp_i[:], pattern=[[1, 1]], base=0, channel_multiplier=1)
            tokp_f = pp.tile([P, 1], f32, tag="tokp_f")
            nc.vector.tensor_copy(tokp_f[:], tokp_i[:])
            coff_t = pp.tile([P, 1], f32, tag="coff_t")
            nc.sync.dma_start(coff_t[:], coff)

            # routing staging [128, NT] (column = token tile)
            d01s = pp.tile([P, NT], f32, tag="d01s")
            idx0s = pp.tile([P, NT], f32, tag="idx0s")
            idx1s = pp.tile([P, NT], f32, tag="idx1s")
            pos0s = pp.tile([P, NT], f32, tag="pos0s")
            pos1s = pp.tile([P, NT], f32, tag="pos1s")

            with tc.tile_pool(name="pxT", bufs=1) as pxp:
                xT = [pxp.tile([P, N], f32r, tag=f"xT_{d}", name=f"xT_{d}") for d in range(DC)]

                # ======== phase B+C: transpose x (fp32 exact), fp32 router ========
                rw_all = pp.tile([P, DC, E], f32, tag="rw_all")
                nc.sync.dma_start(rw_all[:], rw.rearrange("(dc p) e -> p dc e", p=P))
                rwt = [rw_all[:, d, :] for d in range(DC)]
                rbt = pp.tile([E, 1], f32, tag="rbt")
                nc.sync.dma_start(rbt[:], rb)
                with tc.tile_pool(name="pRt", bufs=1) as prt:
                    lgT = prt.tile([E, N], f32, tag="lgT")
                    eq0s, eq1s = [], []
                    with tc.tile_pool(name="pBx", bufs=6) as pbx, \
                         tc.tile_pool(name="pBp", bufs=1, space="PSUM") as pbp:
                        for tg in range(NT // 4):
                            xtiles = []
                            for ti in range(4):
                                xt = pbx.tile([P, D], f32, tag="xload", bufs=4)
                                nc.sync.dma_start(
                                    xt[:], x_r[(tg * 4 + ti) * P:(tg * 4 + ti + 1) * P, :])
                                xtiles.append(xt)
                            xTf = []
                            for d in range(DC):
                                ps = pbp.tile([P, 4 * P], f32, space="PSUM",
                                              tag="ptx", bufs=3)
                                for ti in range(4):
                                    nc.tensor.transpose(
                                        ps[:, ti * P:(ti + 1) * P],
                                        xtiles[ti][:, d * P:(d + 1) * P],
                                        ident[:])
                                nc.scalar.copy(
                                    xT[d][:, tg * 4 * P:(tg + 1) * 4 * P], ps[:])
                                xf = pbx.tile([P, 4 * P], f32, tag="xTf", bufs=8)
                                nc.vector.tensor_copy(xf[:], ps[:])
                                xTf.append(xf)
                            psl = pbp.tile([E, 512], f32, space="PSUM",
                                           tag="ps_lg", bufs=2)
                            for d in range(DC):
                                nc.tensor.matmul(psl[:], rwt[d], xTf[d][:],
                                                 start=(d == 0), stop=(d == DC - 1))
                            nc.scalar.activation(
                                lgT[:, tg * 512:(tg + 1) * 512], psl[:],
                                Act.Identity, bias=rbt[:, 0:1], scale=1.0)

                    # ---- shared expert weights (prefetch, batched DMAs) + act_sh ----
                    swg_all = pp.tile([P, DC, FSH], f32r, tag="swg_all")
                    nc.sync.dma_start(swg_all[:],
                                      swg.rearrange("(dc p) f -> p dc f", p=P))
                    swu_all = pp.tile([P, DC, FSH], f32r, tag="swu_all")
                    nc.sync.dma_start(swu_all[:],
                                      swu.rearrange("(dc p) f -> p dc f", p=P))
                    swdt = []
                    for (f0, fl) in FSH_CH:
                        sd_ = pp.tile([fl, D], f32r, tag=f"swd_{f0}")
                        nc.sync.dma_start(sd_[:], swd[f0:f0 + fl, :])
                        swdt.append(sd_)
                    act_sh = [pp.tile([fl, N], f32r, tag=f"actsh_{f0}", name=f"actsh_{f0}")
                              for (f0, fl) in FSH_CH]

                    # ====== phases D/E: top-2, weights, one-hot ======
                    # (shared-expert gate/up chunks interleaved to keep PE busy)
                    with tc.tile_pool(name="pRt2", bufs=1) as prt2, \
                         tc.tile_pool(name="pRp", bufs=3, space="PSUM") as prp, \
                         tc.tile_pool(name="pSp", bufs=3, space="PSUM") as psp:
                        ohT = prt2.tile([E, N], f32, tag="ohT")
                        cum = prt2.tile([E, N], f32, tag="cum")
                        for t in range(NT):
                            if t % 2 == 0:
                                ci = t // 2
                                fi, n = ci // 4, ci % 4
                                f0, fl = FSH_CH[fi]
                                psg = psp.tile([P, 512], f32, space="PSUM", tag="ps_sh")
                                psu = psp.tile([P, 512], f32, space="PSUM", tag="ps_sh")
                                for d in range(DC):
                                    nc.tensor.matmul(psg[:fl, :], swg_all[:, d, f0:f0 + fl],
                                                     xT[d][:, n * 512:(n + 1) * 512],
                                                     start=(d == 0), stop=(d == DC - 1))
                                for d in range(DC):
                                    nc.tensor.matmul(psu[:fl, :], swu_all[:, d, f0:f0 + fl],
                                                     xT[d][:, n * 512:(n + 1) * 512],
                                                     start=(d == 0), stop=(d == DC - 1))
                                sgact = sp.tile([P, 512], f32r, tag="sgact", bufs=2)
                                nc.scalar.activation(sgact[:fl, :], psg[:fl, :], Act.Silu)
                                nc.vector.tensor_tensor(
                                    out=act_sh[fi][:, n * 512:(n + 1) * 512],
                                    in0=sgact[:fl, :], in1=psu[:fl, :], op=Alu.mult)
                            psl = prp.tile([P, E], f32, space="PSUM", tag="ps_r")
                            nc.tensor.transpose(psl[:], lgT[:, t * P:(t + 1) * P],
                                                ident[:E, :E])
                            lg_t = sp.tile([P, E], f32, tag="lg_t")
                            nc.vector.tensor_copy(lg_t[:], psl[:])
                            mx = sp.tile([P, 8], f32, tag="mx")
                            mi = sp.tile([P, 8], u32, tag="mi")
                            nc.vector.max(mx[:], lg_t[:])
                            nc.vector.max_index(mi[:], mx[:], lg_t[:])
                            nc.vector.tensor_tensor(out=d01s[:, t:t + 1], in0=mx[:, 0:1],
                                                    in1=mx[:, 1:2], op=Alu.subtract)
                            nc.vector.tensor_copy(idx0s[:, t:t + 1], mi[:, 0:1])
                            nc.vector.tensor_copy(idx1s[:, t:t + 1], mi[:, 1:2])
                            mif = sp.tile([P, 2], f32, tag="mif")
                            nc.vector.tensor_copy(mif[:], mi[:, 0:2])
                            eq0 = pp.tile([P, E], f32, tag=f"eq0_{t}")
                            eq1 = pp.tile([P, E], f32, tag=f"eq1_{t}")
                            nc.vector.tensor_tensor(out=eq0[:], in0=iota_f[:],
                                                    in1=mif[:, 0:1].to_broadcast([P, E]),
                                                    op=Alu.is_equal)
                            nc.vector.tensor_tensor(out=eq1[:], in0=iota_f[:],
                                                    in1=mif[:, 1:2].to_broadcast([P, E]),
                                                    op=Alu.is_equal)
                            eq0s.append(eq0)
                            eq1s.append(eq1)
                            oh = sp.tile([P, E], f32, tag="oh")
                            nc.vector.tensor_add(oh[:], eq0[:], eq1[:])
                            pso = prp.tile([E, P], f32, space="PSUM", tag="ps_r")
                            nc.tensor.transpose(pso[:], oh[:], ident[:])
                            nc.scalar.copy(ohT[:, t * P:(t + 1) * P], pso[:])

                        w0s = pp.tile([P, NT], f32, tag="w0s")
                        w1s = pp.tile([P, NT], f32, tag="w1s")
                        nc.scalar.activation(w0s[:], d01s[:], Act.Sigmoid)
                        nc.vector.tensor_scalar(out=w1s[:], in0=w0s[:], scalar1=-1.0,
                                                scalar2=1.0, op0=Alu.mult, op1=Alu.add)

                        # ====== phase F: cumulative counts (inclusive) ======
                        zcol = pp.tile([P, 1], f32, tag="zcol")
                        nc.vector.memset(zcol[:], 0.0)
                        nc.vector.tensor_tensor_scan(
                            cum[:], ohT[:], zcol[:E, 0:1].to_broadcast([E, N]), 0.0,
                            op0=Alu.add, op1=Alu.add)

                        if stage >= 2:
                            # ====== shared down -> base partial (fills PE
                            # while DVE runs cumsum/positions below) ======
                            for t in range(NT):
                                ysh = sp.tile([P, D], f32, tag="big4k", bufs=4,
                                              name="ysh")
                                for n in range(2):
                                    psd = psp.tile([P, 512], f32, space="PSUM",
                                                   tag="ps_sh")
                                    for fi, (f0, fl) in enumerate(FSH_CH):
                                        nc.tensor.matmul(
                                            psd[:], act_sh[fi][:, t * P:(t + 1) * P],
                                            swdt[fi][:, n * 512:(n + 1) * 512],
                                            start=(fi == 0),
                                            stop=(fi == len(FSH_CH) - 1))
                                    nc.scalar.copy(ysh[:, n * 512:(n + 1) * 512],
                                                   psd[:])
                                nc.sync.dma_start(partial[t * P:(t + 1) * P, :],
                                                  ysh[:])

                        # ====== phase G: per-slot positions ======
                        for t in range(NT):
                            psc = prp.tile([P, E], f32, space="PSUM", tag="ps_r")
                            nc.tensor.transpose(psc[:], cum[:, t * P:(t + 1) * P],
                                                ident[:E, :E])
                            cumP = sp.tile([P, E], f32, tag="cumP")
                            nc.vector.tensor_copy(cumP[:], psc[:])
                            scr = sp.tile([P, E], f32, tag="scr")
                            nc.vector.tensor_mul(scr[:], eq0s[t][:], cumP[:])
                            nc.vector.reduce_sum(pos0s[:, t:t + 1], scr[:],
                                                 axis=mybir.AxisListType.X)
                            scr2 = sp.tile([P, E], f32, tag="scr2")
                            nc.vector.tensor_mul(scr2[:], eq1s[t][:], cumP[:])
                            nc.vector.reduce_sum(pos1s[:, t:t + 1], scr2[:],
                                                 axis=mybir.AxisListType.X)

                # ====== phase H: slot indices (f32; TRASH=NSLOT matches no slot) ======
                slotf = []
                for kk, (idxs, poss) in enumerate(((idx0s, pos0s), (idx1s, pos1s))):
                    loc = pp.tile([P, NT], f32, tag=f"loc{kk}")
                    nc.vector.tensor_scalar(out=loc[:], in0=idxs[:],
                                            scalar1=coff_t[:, 0:1], scalar2=None,
                                            op0=Alu.subtract)
                    pm1 = pp.tile([P, NT], f32, tag=f"pm1{kk}")
                    nc.vector.tensor_scalar_add(pm1[:], poss[:], -1.0)
                    v1 = sp.tile([P, NT], f32, tag="v1")
                    nc.vector.tensor_scalar(out=v1[:], in0=loc[:], scalar1=-0.5,
                                            scalar2=None, op0=Alu.is_gt)
                    v2 = sp.tile([P, NT], f32, tag="v2")
                    nc.vector.tensor_scalar(out=v2[:], in0=loc[:],
                                            scalar1=float(EPC) - 0.5, scalar2=None,
                                            op0=Alu.is_lt)
                    v3 = sp.tile([P, NT], f32, tag="v3")
                    nc.vector.tensor_scalar(out=v3[:], in0=pm1[:],
                                            scalar1=float(C) - 0.5, scalar2=None,
                                            op0=Alu.is_lt)
                    val = pp.tile([P, NT], f32, tag=f"val{kk}")
                    nc.vector.tensor_mul(val[:], v1[:], v2[:])
                    nc.vector.tensor_mul(val[:], val[:], v3[:])
                    gf = pp.tile([P, NT], f32, tag=f"gf{kk}")
                    nc.vector.tensor_scalar(out=gf[:], in0=loc[:], scalar1=float(CP),
                                            scalar2=None, op0=Alu.mult)
                    nc.vector.tensor_add(gf[:], gf[:], pm1[:])
                    nc.vector.tensor_scalar_add(gf[:], gf[:], -float(TRASH))
                    nc.vector.tensor_mul(gf[:], gf[:], val[:])
                    nc.vector.tensor_scalar_add(gf[:], gf[:], float(TRASH))
                    slotf.append(gf)

                if debug:
                    for t in range(NT):
                        dbt = sp.tile([P, 8], f32, tag="dbt")
                        for j, src in enumerate((idx0s, idx1s, pos0s, pos1s, w0s, w1s)):
                            nc.vector.tensor_copy(dbt[:, j:j + 1], src[:, t:t + 1])
                        nc.vector.tensor_copy(dbt[:, 6:7], slotf[0][:, t:t + 1])
                        nc.vector.tensor_copy(dbt[:, 7:8], slotf[1][:, t:t + 1])
                        nc.sync.dma_start(dbg_route[t * P:(t + 1) * P, :], dbt[:])

                if stage >= 1:
                    # ====== phase I: slot tables via one-hot matmuls ======
                    # mt[p, s] = (slot_k(token) == s); psum rows accumulate
                    # (tile_base, lane, weight) per slot. Token id split keeps
                    # every operand exactly representable at low precision.
                    toki, wcol = [], []
                    with tc.tile_pool(name="pTbl", bufs=1) as ptl, \
                         tc.tile_pool(name="ptb", bufs=1, space="PSUM") as ptb:
                        iota_s_i = ptl.tile([P, NSLOT], i32, tag="iota_s_i")
                        nc.gpsimd.iota(iota_s_i[:], pattern=[[1, NSLOT]], base=0,
                                       channel_multiplier=0)
                        iota_s = ptl.tile([P, NSLOT], f16, tag="iota_s")
                        nc.vector.tensor_copy(iota_s[:], iota_s_i[:])
                        # token ids (p + 128t <= 2047: exact in f16)
                        tok_i = ptl.tile([P, NT], i32, tag="tok_i")
                        nc.gpsimd.iota(tok_i[:], pattern=[[P, NT]], base=0,
                                       channel_multiplier=1)
                        pw = []
                        for kk, ws in ((0, w0s), (1, w1s)):
                            pwk = ptl.tile([P, NT, 2], f16, tag=f"pw{kk}",
                                           name=f"pw{kk}")
                            nc.vector.tensor_copy(pwk[:, :, 0:1], tok_i[:])
                            nc.vector.tensor_copy(pwk[:, :, 1:2], ws[:])
                            pw.append(pwk)
                        tbl_ps = [ptb.tile([2, CP], f32, space="PSUM",
                                           tag=f"tbl_ps{h}", name=f"tbl_ps{h}")
                                  for h in range(2)]
                        for t in range(NT):
                            for kk in range(2):
                                mt = ptl.tile([P, NSLOT], f16, tag="mt", bufs=3)
                                nc.vector.tensor_scalar(
                                    out=mt[:], in0=iota_s[:],
                                    scalar1=slotf[kk][:, t:t + 1], scalar2=None,
                                    op0=Alu.is_equal)
                                first = (t == 0 and kk == 0)
                                last = (t == NT - 1 and kk == 1)
                                for h in range(2):
                                    nc.tensor.matmul(
                                        tbl_ps[h][:], pw[kk][:, t, :],
                                        mt[:, h * CP:(h + 1) * CP],
                                        start=first, stop=last)
                        tbl_sb = ptl.tile([2, NSLOT], f32, tag="tbl_sb")
                        for h in range(2):
                            nc.scalar.copy(tbl_sb[:, h * CP:(h + 1) * CP],
                                           tbl_ps[h][:])
                        for c in range(ST):
                            pst = ptb.tile([P, 2], f32, space="PSUM", tag="pst",
                                           bufs=2)
                            nc.tensor.transpose(
                                pst[:], tbl_sb[:, c * P:(c + 1) * P],
                                ident[:2, :2])
                            pstv = ptl.tile([P, 2], f32, tag="pstv", bufs=2)
                            nc.vector.tensor_copy(pstv[:], pst[:])
                            tci = pp.tile([P, 1], i32, tag=f"toki_{c}")
                            nc.vector.tensor_copy(tci[:], pstv[:, 0:1])
                            toki.append(tci)
                            wc = pp.tile([P, 1], f32, tag=f"wcol_{c}")
                            nc.vector.tensor_copy(wc[:], pstv[:, 1:2])
                            wcol.append(wc)
                            if debug:
                                dtb = ptl.tile([P, 2], f32, tag="dtb", bufs=2)
                                nc.vector.tensor_copy(dtb[:], pstv[:])
                                nc.sync.dma_start(
                                    dbg_tbl[c * P:(c + 1) * P, :], dtb[:])



            if stage >= 3:
                    # ====== dispatch: gather tokens, transpose to hT ======
                hT = [pp.tile([P, NSLOT], bf16, tag=f"hT_{d}", name=f"hT_{d}") for d in range(DC)]
                with tc.tile_pool(name="pthp", bufs=2, space="PSUM") as pth:
                    for c in range(ST):
                        hc = sp.tile([P, D], f32r, tag="big4k", bufs=4, name="hc")
                        nc.gpsimd.indirect_dma_start(
                            out=hc[:], out_offset=None, in_=x_r,
                            in_offset=bass.IndirectOffsetOnAxis(ap=toki[c][:, 0:1], axis=0))
                        for d in range(DC):
                            psh = pth.tile([P, P], f32r, space="PSUM", tag="psh")
                            nc.tensor.transpose(psh[:], hc[:, d * P:(d + 1) * P], ident_r[:])
                            nc.scalar.copy(hT[d][:, c * P:(c + 1) * P], psh[:])

            if stage >= 4:
                    # ====== expert MLPs + combine ======
                with tc.tile_pool(name="wpool", bufs=1) as wp, \
                     tc.tile_pool(name="pact", bufs=11) as pact, \
                     tc.tile_pool(name="pEp", bufs=2, space="PSUM") as pep:
                    for e in range(EPC):
                        # batched weight DMAs: one per matrix group
                        wgas, wuas = [], []
                        for (fg0, fgn) in FG:
                            wga = wp.tile([P, DC, fgn * P], bf16,
                                          tag=f"wga_{fg0}", bufs=2, name="wga")
                            nc.sync.dma_start(
                                wga[:], wg[e, :, fg0 * P:(fg0 + fgn) * P]
                                .rearrange("(dc p) f -> p dc f", p=P))
                            wgas.append(wga)
                            wua = wp.tile([P, DC, fgn * P], bf16,
                                          tag=f"wua_{fg0}", bufs=2, name="wua")
                            nc.sync.dma_start(
                                wua[:], wu[e, :, fg0 * P:(fg0 + fgn) * P]
                                .rearrange("(dc p) f -> p dc f", p=P))
                            wuas.append(wua)
                        wda = wp.tile([P, FC, D], bf16, tag="wda", bufs=1,
                                      name="wda")
                        nc.sync.dma_start(
                            wda[:], wd[e].rearrange("(fc p) dd -> p fc dd", p=P))
                        acts = []
                        for gidx, (fg0, fgn) in enumerate(FG):
                            for fi in range(fgn):
                                psg = pep.tile([P, CP], f32, space="PSUM", tag="ps_eg")
                                psu = pep.tile([P, CP], f32, space="PSUM", tag="ps_eu")
                                for d in range(DC):
                                    nc.tensor.matmul(
                                        psg[:], wgas[gidx][:, d, fi * P:(fi + 1) * P],
                                        hT[d][:, e * CP:(e + 1) * CP],
                                        start=(d == 0), stop=(d == DC - 1))
                                for d in range(DC):
                                    nc.tensor.matmul(
                                        psu[:], wuas[gidx][:, d, fi * P:(fi + 1) * P],
                                        hT[d][:, e * CP:(e + 1) * CP],
                                        start=(d == 0), stop=(d == DC - 1))
                                sgact = sp.tile([P, CP], bf16, tag="esg", bufs=2)
                                nc.scalar.activation(sgact[:], psg[:], Act.Silu)
                                af = pact.tile([P, CP], bf16, tag="act")
                                nc.vector.tensor_tensor(out=af[:], in0=sgact[:],
                                                        in1=psu[:], op=Alu.mult)
                                acts.append(af)
                        # down-projection per slot block, scatter-add right away
                        for c in range(3):
                            yt = sp.tile([P, D], f32, tag="ycomb", bufs=3,
                                         name="yt")
                            for n in range(2):
                                psy = pep.tile([P, 512], f32, space="PSUM", tag="ps_ey")
                                for f in range(FC):
                                    nc.tensor.matmul(
                                        psy[:], acts[f][:, c * P:(c + 1) * P],
                                        wda[:, f, n * 512:(n + 1) * 512],
                                        start=(f == 0), stop=(f == FC - 1))
                                nc.vector.tensor_scalar(
                                    out=yt[:, n * 512:(n + 1) * 512], in0=psy[:],
                                    scalar1=wcol[e * 3 + c][:, 0:1], scalar2=None,
                                    op0=Alu.mult)
                            nc.gpsimd.indirect_dma_start(
                                out=partial,
                                out_offset=bass.IndirectOffsetOnAxis(
                                    ap=toki[e * 3 + c][:, 0:1], axis=0),
                                in_=yt[:], in_offset=None,
                                bounds_check=N - 1, oob_is_err=False,
                                compute_op=Alu.add)

    return _finish(nc)


def _finish(nc):
    nc.compile()
    return nc


def _get_nc(debug=False, stage=99):
    key = ("nc", debug, stage)
    if key not in _BUILD_CACHE:
        _BUILD_CACHE[key] = _build_nc(debug, stage)
    return _BUILD_CACHE[key]


def kernel(x, router_w, router_b, w_gate, w_up, w_down, sw_gate, sw_up, sw_down,
           _debug=False, _trace=False):
    from concourse.bass_utils import run_bass_kernel_spmd

    import ml_dtypes
    bf16 = ml_dtypes.bfloat16

    x = np.asarray(x, np.float32)
    x2 = np.ascontiguousarray(x.reshape(N, D))
    wg_b = np.asarray(w_gate, np.float32).astype(bf16)
    wu_b = np.asarray(w_up, np.float32).astype(bf16)
    wd_b = np.asarray(w_down, np.float32).astype(bf16)
    in_maps = []
    for m in range(NCORES):
        fs = slice(m * FSH, (m + 1) * FSH)
        in_maps.append({
            "x_r": x2,
            "rw": np.ascontiguousarray(np.asarray(router_w, np.float32)),
            "rb": np.ascontiguousarray(np.asarray(router_b, np.float32).reshape(E, 1)),
            "wg": np.ascontiguousarray(wg_b[m * EPC:(m + 1) * EPC]),
            "wu": np.ascontiguousarray(wu_b[m * EPC:(m + 1) * EPC]),
            "wd": np.ascontiguousarray(wd_b[m * EPC:(m + 1) * EPC]),
            "swg": np.ascontiguousarray(np.asarray(sw_gate, np.float32)[:, fs]),
            "swu": np.ascontiguousarray(np.asarray(sw_up, np.float32)[:, fs]),
            "swd": np.ascontiguousarray(np.asarray(sw_down, np.float32)[fs, :]),
            "coff": np.full((P, 1), float(m * EPC), np.float32),
        })

    nc = _get_nc(_debug)
    res = run_bass_kernel_spmd(nc, in_maps, core_ids=list(range(NCORES)),
                               trace=_trace)
    out = x2.copy()
    for r in res.results:
        out += r["partial"]
    if _debug or _trace:
        kernel._last_results = res
    return out.reshape(x.shape)

